# revision 1
# baseline (speedup 1.0000x reference)
"""3-layer LSTM (B=256,T=512,I=256,H=512) + linear head on 8 NeuronCores.

Strategy: data-parallel over batch (32/core). Per layer, the input-side
matmul G = Wih @ x_t (+ biases) for a *chunk* of future time steps is
computed at full PE efficiency (N=512 streams) and interleaved with the
sequential h-recurrence of the current chunk; G never leaves SBUF.
Gate layout: gates.T packed [128 part, 512 cols] = 16 slots of 32 batch
cols in slot order i|f|o|g, all in ONE PSUM bank per step, preloaded
with G via one DVE copy, accumulated by 64 weight-stationary bf16
matmuls (K=128, M=128, N=32), then 2 ACT instrs (sigmoid over i|f|o,
tanh over g) evacuate to SBUF. c stays fp32-resident; h is written
bf16 directly into the layout the next matmul and the next layer's
batched input matmul consume.
"""

import numpy as np
import ml_dtypes
from contextlib import ExitStack

import concourse.bass as bass
import concourse.bacc as bacc
import concourse.tile as tile
from concourse import mybir
from concourse.bass_utils import run_bass_kernel_spmd

BF16 = mybir.dt.bfloat16
F32 = mybir.dt.float32
AF = mybir.ActivationFunctionType

B, T, I, H, O = 256, 512, 256, 512, 3
NCORES = 8
BL = B // NCORES          # 32 batch rows per core
SC = 16                   # time steps per chunk
CW = SC * BL              # 512 cols per chunk
NCH = T // SC             # 32 chunks
TOT = T * BL              # 16384 cols total
SLACK = 2 * CW            # prefetch overrun slack (cols)

# gate blocks in psum-slot order: i | f | o | g  (slot = blk*4 + j)
# block -> base row in the canonical (i,f,g,o) 2048 gate layout
GATE_BASE = [0, 512, 1536, 1024]   # i, f, o, g
KCS = [2, 4, 4]                    # K chunks per layer (256, 512, 512)


def _slot_row(slot):
    return GATE_BASE[slot // 4] + 128 * (slot % 4)


def _build():
    nc = bacc.Bacc("TRN2", target_bir_lowering=False, debug=False,
                   num_devices=NCORES)

    xt = nc.dram_tensor("x_t", (128, 2, TOT + SLACK), BF16, kind="ExternalInput")
    wih = [nc.dram_tensor(f"wih{l}", (128, KCS[l] * 2048), BF16,
                          kind="ExternalInput") for l in range(3)]
    whh = [nc.dram_tensor(f"whh{l}", (128, 4 * 2048), BF16,
                          kind="ExternalInput") for l in range(3)]
    bias_d = nc.dram_tensor("bias", (128, 48), F32, kind="ExternalInput")
    fcw_d = nc.dram_tensor("fcw", (128, 12), BF16, kind="ExternalInput")
    fcb_d = nc.dram_tensor("fcb", (3, 1), F32, kind="ExternalInput")
    out_d = nc.dram_tensor("out", (3, BL), F32, kind="ExternalOutput")

    with tile.TileContext(nc) as tc, ExitStack() as ctx:
        dram = ctx.enter_context(tc.tile_pool(name="dram", bufs=1, space="DRAM"))
        hdr = dram.tile([128, 4, TOT + SLACK], BF16)   # inter-layer H seq

        const = ctx.enter_context(tc.tile_pool(name="const", bufs=1))
        wih_sb = [const.tile([128, KCS[l] * 2048], BF16, tag=f"wih{l}",
                             name=f"wih_sb{l}") for l in range(3)]
        whh_sb = [const.tile([128, 4 * 2048], BF16, tag=f"whh{l}",
                             name=f"whh_sb{l}") for l in range(3)]
        bias_sb = const.tile([128, 48], F32, tag="bias")
        fcw_sb = const.tile([128, 12], BF16, tag="fcw")
        fcb_sb = const.tile([3, 1], F32, tag="fcb")
        for l in range(3):
            nc.sync.dma_start(wih_sb[l][:], wih[l].ap())
            nc.sync.dma_start(whh_sb[l][:], whh[l].ap())
        nc.sync.dma_start(bias_sb[:], bias_d.ap())
        nc.sync.dma_start(fcw_sb[:], fcw_d.ap())
        nc.sync.dma_start(fcb_sb[:], fcb_d.ap())

        big = ctx.enter_context(tc.tile_pool(name="big", bufs=1))
        g_buf = big.tile([128, 2 * 16 * CW], BF16, tag="gbuf")     # 4MB
        in_buf = big.tile([128, 4 * 4 * CW], BF16, tag="inbuf")    # 2MB
        h_stage = big.tile([128, 2 * 4 * CW], BF16, tag="hstage")  # 1MB
        c_t = big.tile([128, 128], F32, tag="cstate")

        g3 = g_buf[:].rearrange("p (s c) -> p s c", c=CW)    # [128, 32, CW]
        i3 = in_buf[:].rearrange("p (b c) -> p b c", c=CW)   # [128, 16, CW]
        h3 = h_stage[:].rearrange("p (x c) -> p x c", c=CW)  # [128, 8, CW]

        ew = ctx.enter_context(tc.tile_pool(name="ew", bufs=2))
        ps_rec = ctx.enter_context(tc.tile_pool(name="psr", bufs=2, space="PSUM"))
        ps_pa = ctx.enter_context(tc.tile_pool(name="psa", bufs=2, space="PSUM"))
        ps_fc = ctx.enter_context(tc.tile_pool(name="psf", bufs=1, space="PSUM"))

        def phase_a_slot(l, slot, in_base, g_base, in_ap):
            """G[slot] for one chunk: Kc matmuls (N=CW) + bias ACT."""
            kc = KCS[l]
            ps = ps_pa.tile([128, CW], F32, tag="pa")
            for k in range(kc):
                nc.tensor.matmul(
                    ps[:],
                    lhsT=wih_sb[l][:, k * 2048 + _slot_row(slot):
                                   k * 2048 + _slot_row(slot) + 128],
                    rhs=in_ap(in_base + k),
                    start=(k == 0), stop=(k == kc - 1),
                )
            nc.scalar.activation(
                g3[:, bass.ds(g_base + slot, 1), :].rearrange("p a c -> p (a c)"),
                ps[:], AF.Identity, bias=bias_sb[:, l * 16 + slot: l * 16 + slot + 1])

        def rec_step(l, s, g_base, h_rd, h_wr, pa_emit):
            """One recurrence time step; h_rd/h_wr are h3 block bases."""
            ps = ps_rec.tile([128, 512], F32, tag="rec")
            nc.vector.tensor_copy(
                ps[:].rearrange("p (a b) -> p a b", b=BL),
                g3[:, bass.ds(g_base, 16), s * BL: (s + 1) * BL])
            # h[t-1]: last slot of the other-parity buffer for s=0,
            # else slot s-1 of the current chunk's buffer
            hp_base = h_rd if s == 0 else h_wr
            hp_col = ((SC - 1) if s == 0 else (s - 1)) * BL
            for slot in range(16):
                for k in range(4):
                    nc.tensor.matmul(
                        ps[:, slot * BL:(slot + 1) * BL],
                        lhsT=whh_sb[l][:, k * 2048 + _slot_row(slot):
                                       k * 2048 + _slot_row(slot) + 128],
                        rhs=h3[:, bass.ds(hp_base + k, 1),
                               hp_col:hp_col + BL].rearrange("p a c -> p (a c)"),
                        start=False, stop=(k == 3), skip_group_check=True,
                    )
            gt = ew.tile([128, 512], F32, tag="gates")
            nc.scalar.activation(gt[:, 0:384], ps[:, 0:384], AF.Sigmoid)
            nc.scalar.activation(gt[:, 384:512], ps[:, 384:512], AF.Tanh)
            t1 = ew.tile([128, 128], F32, tag="t1")
            t2 = ew.tile([128, 128], F32, tag="t2")
            nc.vector.tensor_mul(t1[:], gt[:, 0:128], gt[:, 384:512])    # i*g
            nc.vector.tensor_mul(t2[:], gt[:, 128:256], c_t[:])          # f*c
            nc.vector.tensor_add(c_t[:], t1[:], t2[:])
            th = ew.tile([128, 128], F32, tag="th")
            nc.scalar.activation(th[:], c_t[:], AF.Tanh)
            nc.vector.tensor_mul(
                h3[:, bass.ds(h_wr, 4), s * BL:(s + 1) * BL],
                gt[:, 256:384].rearrange("p (a b) -> p a b", b=BL),
                th[:].rearrange("p (a b) -> p a b", b=BL))
            if pa_emit is not None:
                pa_emit(s)

        for l in range(3):
            in_dram = xt.ap() if l == 0 else hdr[:]
            kc = KCS[l]

            # prologue: In chunks 0,1 -> bufs 0,1 ; G chunk 0 -> parity 0
            nc.sync.dma_start(i3[:, 0:kc, :], in_dram[:, :, 0:CW])
            nc.sync.dma_start(i3[:, kc:2 * kc, :], in_dram[:, :, CW:2 * CW])
            for slot in range(16):
                phase_a_slot(l, slot, 0, 0,
                             lambda idx: i3[:, bass.ds(idx, 1), :]
                             .rearrange("p a c -> p (a c)"))
            nc.vector.memset(c_t[:], 0.0)
            nc.vector.memset(h3[:, bass.ds(4, 4), (SC - 1) * BL: SC * BL], 0.0)

            def body(iv, l=l, kc=kc, in_dram=in_dram):
                p2 = iv & 1
                q2 = (iv + 1) & 1
                ld_buf = ((iv + 2) & 3) * kc
                use_buf = ((iv + 1) & 3) * kc
                nc.sync.dma_start(
                    i3[:, bass.ds(ld_buf, kc), :],
                    in_dram[:, :, bass.ds((iv + 2) * CW, CW)])

                def pa_emit(s, l=l, use_buf=use_buf, q2=q2):
                    phase_a_slot(l, s, use_buf, q2 * 16,
                                 lambda idx: i3[:, bass.ds(idx, 1), :]
                                 .rearrange("p a c -> p (a c)"))

                for s in range(SC):
                    rec_step(l, s, p2 * 16, q2 * 4, p2 * 4, pa_emit)
                if l < 2:
                    nc.sync.dma_start(
                        hdr[:, :, bass.ds(iv * CW, CW)],
                        h3[:, bass.ds(p2 * 4, 4), :])

            with tc.For_i(0, NCH, 1) as iv:
                body(iv)

        # final linear head: out.T [3, BL] = fcW @ h_last (+ fcB)
        hb = ((NCH - 1) & 1) * 4
        ps = ps_fc.tile([3, BL], F32, tag="fc")
        for k in range(4):
            nc.tensor.matmul(
                ps[:], lhsT=fcw_sb[:, k * 3:(k + 1) * 3],
                rhs=h3[:, bass.ds(hb + k, 1), (SC - 1) * BL: SC * BL]
                .rearrange("p a c -> p (a c)"),
                start=(k == 0), stop=(k == 3))
        ob = ew.tile([3, BL], F32, tag="out")
        nc.scalar.activation(ob[:], ps[:], AF.Identity, bias=fcb_sb[:])
        nc.sync.dma_start(out_d.ap(), ob[:])

    nc.compile()
    return nc


def _prep(inputs):
    """Host-side layout prep. Returns per-core in_maps."""
    bf = ml_dtypes.bfloat16
    x = np.asarray(inputs["x"], np.float32)
    wihs = [np.asarray(inputs[f"Wih{l}"], np.float32) for l in range(3)]
    whhs = [np.asarray(inputs[f"Whh{l}"], np.float32) for l in range(3)]

    def wt_pack(w, kcs):  # [2048, K] -> [128, kcs*2048]
        return np.ascontiguousarray(
            w.T.reshape(kcs, 128, 2048).transpose(1, 0, 2)
            .reshape(128, kcs * 2048)).astype(bf)

    shared = {}
    for l in range(3):
        shared[f"wih{l}"] = wt_pack(wihs[l], KCS[l])
        shared[f"whh{l}"] = wt_pack(whhs[l], 4)
    bias = np.zeros((128, 48), np.float32)
    for l in range(3):
        bl_ = (np.asarray(inputs[f"bih{l}"], np.float32)
               + np.asarray(inputs[f"bhh{l}"], np.float32))
        for slot in range(16):
            r = _slot_row(slot)
            bias[:, l * 16 + slot] = bl_[r:r + 128]
    shared["bias"] = bias
    shared["fcw"] = np.ascontiguousarray(
        np.asarray(inputs["fcW"], np.float32).T.reshape(4, 128, 3)
        .transpose(1, 0, 2).reshape(128, 12)).astype(bf)
    shared["fcb"] = np.asarray(inputs["fcB"], np.float32).reshape(3, 1)

    in_maps = []
    for c in range(NCORES):
        xc = x[c * BL:(c + 1) * BL]                       # [32, 512, 256]
        xp = xc.transpose(2, 1, 0).reshape(2, 128, TOT)   # [2,128,16384]
        xp = np.ascontiguousarray(xp.transpose(1, 0, 2))  # [128,2,16384]
        xp = np.concatenate(
            [xp, np.zeros((128, 2, SLACK), np.float32)], axis=2).astype(bf)
        in_maps.append({"x_t": xp, **shared})
    return in_maps


_NC_CACHE = None


def kernel(**inputs):
    global _NC_CACHE
    if _NC_CACHE is None:
        _NC_CACHE = _build()
    nc = _NC_CACHE
    in_maps = _prep(inputs)
    res = run_bass_kernel_spmd(nc, in_maps, core_ids=list(range(NCORES)))
    out = np.empty((B, O), np.float32)
    for c in range(NCORES):
        out[c * BL:(c + 1) * BL] = res.results[c]["out"].T
    return out



# revision 4
# speedup vs baseline: 1.0236x; 1.0236x over previous
"""3-layer LSTM (B=256,T=512,I=256,H=512) + linear head on 8 NeuronCores.

v2: static access patterns everywhere on the compute engines.

Strategy: data-parallel over batch (32/core). Per layer, the input-side
matmul G = Wih @ x_t (+ biases) for a chunk of 16 future time steps is
computed at N=512 PE efficiency and interleaved with the sequential
h-recurrence; G never leaves SBUF. The chunk loop is a hardware For_i
unrolled 4x so every SBUF/PSUM access pattern is compile-time static
(dynamic register APs serialize the PE: ~290ns/matmul vs ~36ns static).

Per step the 16 gate slots are split into two hidden-dim halves, each
with its own PSUM bank, so the sigmoid/tanh + elementwise chain of half
A runs on ACT/DVE while the PE still accumulates half B; K-chunks 0,1
are swept first so the next step's matmuls can begin as soon as half
A's h (hidden dims 0:255) is written.

Slot order (v2): per half h in {0,1}: [i,i,f,f,o,o,g,g] with hidden
rows 2h*128 and (2h+1)*128 - psum half-bank layout i|f|o|g of 64 cols
each, so one sigmoid covers i,f,o and one tanh covers g.
"""

import numpy as np
import ml_dtypes
from contextlib import ExitStack

import concourse.bass as bass
import concourse.bacc as bacc
import concourse.tile as tile
from concourse import mybir
from concourse.bass_utils import run_bass_kernel_spmd

BF16 = mybir.dt.bfloat16
F32 = mybir.dt.float32
AF = mybir.ActivationFunctionType

B, T, I, H, O = 256, 512, 256, 512, 3
NCORES = 8
BL = B // NCORES          # 32 batch rows per core
SC = 16                   # time steps per chunk
CW = SC * BL              # 512 cols per chunk
NCH = T // SC             # 32 chunks
TOT = T * BL              # 16384 cols total
SLACK = 2 * CW            # prefetch overrun slack (cols)

KCS = [2, 4, 4]                    # K chunks per layer (256, 512, 512)
# block base row in canonical (i,f,g,o) 2048-row gate layout
GB = {"i": 0, "f": 512, "g": 1024, "o": 1536}
# v2 slot order: half h: [i,i,f,f,o,o,g,g], hidden idx 2h + (pos&1)
SLOT_BLOCKS = ["i", "i", "f", "f", "o", "o", "g", "g"]


def _slot_row(slot):
    half, pos = slot // 8, slot % 8
    return GB[SLOT_BLOCKS[pos]] + (2 * half + (pos & 1)) * 128


def _build():
    nc = bacc.Bacc("TRN2", target_bir_lowering=False, debug=False,
                   num_devices=NCORES)

    xt = nc.dram_tensor("x_t", (128, 2, TOT + SLACK), BF16, kind="ExternalInput")
    wih = [nc.dram_tensor(f"wih{l}", (128, KCS[l] * 2048), BF16,
                          kind="ExternalInput") for l in range(3)]
    whh = [nc.dram_tensor(f"whh{l}", (128, 4 * 2048), BF16,
                          kind="ExternalInput") for l in range(3)]
    bias_d = nc.dram_tensor("bias", (128, 48), F32, kind="ExternalInput")
    fcw_d = nc.dram_tensor("fcw", (128, 12), BF16, kind="ExternalInput")
    fcb_d = nc.dram_tensor("fcb", (3, 1), F32, kind="ExternalInput")
    out_d = nc.dram_tensor("out", (3, BL), F32, kind="ExternalOutput")

    with tile.TileContext(nc) as tc, ExitStack() as ctx:
        dram = ctx.enter_context(tc.tile_pool(name="dram", bufs=1, space="DRAM"))
        hdr = dram.tile([128, 4, TOT + SLACK], BF16)   # inter-layer H seq

        const = ctx.enter_context(tc.tile_pool(name="const", bufs=1))
        wih_sb = [const.tile([128, KCS[l] * 2048], BF16, tag=f"wih{l}",
                             name=f"wih_sb{l}") for l in range(3)]
        whh_sb = [const.tile([128, 4 * 2048], BF16, tag=f"whh{l}",
                             name=f"whh_sb{l}") for l in range(3)]
        bias_sb = const.tile([128, 48], F32, tag="bias")
        fcw_sb = const.tile([128, 12], BF16, tag="fcw")
        fcb_sb = const.tile([3, 1], F32, tag="fcb")
        for l in range(3):
            nc.sync.dma_start(wih_sb[l][:], wih[l].ap())
            nc.sync.dma_start(whh_sb[l][:], whh[l].ap())
        nc.sync.dma_start(bias_sb[:], bias_d.ap())
        nc.sync.dma_start(fcw_sb[:], fcw_d.ap())
        nc.sync.dma_start(fcb_sb[:], fcb_d.ap())

        big = ctx.enter_context(tc.tile_pool(name="big", bufs=1))
        g_buf = big.tile([128, 2 * 16 * CW], BF16, tag="gbuf")     # 4MB
        in_buf = big.tile([128, 4 * 4 * CW], BF16, tag="inbuf")    # 2MB
        h_stage = big.tile([128, 2 * 4 * CW], BF16, tag="hstage")  # 1MB
        c_t = big.tile([128, 128], F32, tag="cstate")

        g3 = g_buf[:].rearrange("p (s c) -> p s c", c=CW)    # [128, 32, CW]
        i3 = in_buf[:].rearrange("p (b c) -> p b c", c=CW)   # [128, 16, CW]
        h3 = h_stage[:].rearrange("p (x c) -> p x c", c=CW)  # [128, 8, CW]

        ew = ctx.enter_context(tc.tile_pool(name="ew", bufs=2))
        ps_rec = ctx.enter_context(tc.tile_pool(name="psr", bufs=2, space="PSUM"))
        ps_pa = ctx.enter_context(tc.tile_pool(name="psa", bufs=3, space="PSUM"))
        ps_fc = ctx.enter_context(tc.tile_pool(name="psf", bufs=1, space="PSUM"))

        # rolling pair of rec psum tiles (one full bank per half)
        state = {"ps": None}

        def alloc_preload(parity, s):
            """Allocate next step's psum pair and preload G into cols 0:256."""
            ps = [ps_rec.tile([128, 512], F32, tag=f"rec{h}", name=f"ps_rec{h}")
                  for h in (0, 1)]
            for h in (0, 1):
                nc.vector.tensor_copy(
                    ps[h][:, 0:256].rearrange("p (a b) -> p a b", b=BL),
                    g3[:, bass.ds(parity * 16 + h * 8, 8),
                       s * BL:(s + 1) * BL])
            return ps

        def phase_a_mms(l, slot, in_base, in_ap):
            """G[slot] matmuls for one chunk (kc matmuls, N=CW)."""
            kc = KCS[l]
            ps = ps_pa.tile([128, CW], F32, tag="pa")
            for k in range(kc):
                nc.tensor.matmul(
                    ps[:],
                    lhsT=wih_sb[l][:, k * 2048 + _slot_row(slot):
                                   k * 2048 + _slot_row(slot) + 128],
                    rhs=in_ap(in_base + k),
                    start=(k == 0), stop=(k == kc - 1),
                )
            return ps

        def phase_a_evac(ps, l, slot, g_base):
            nc.scalar.activation(
                g3[:, bass.ds(g_base + slot, 1), :].rearrange("p a c -> p (a c)"),
                ps[:], AF.Identity, bias=bias_sb[:, l * 16 + slot: l * 16 + slot + 1])

        def phase_a_slot(l, slot, in_base, g_base, in_ap):
            phase_a_evac(phase_a_mms(l, slot, in_base, in_ap), l, slot, g_base)

        def rec_mm(l, ps, half, pos, ks, hp_base, hp_col):
            slot = half * 8 + pos
            for k in ks:
                nc.tensor.matmul(
                    ps[half][:, pos * BL:(pos + 1) * BL],
                    lhsT=whh_sb[l][:, k * 2048 + _slot_row(slot):
                                   k * 2048 + _slot_row(slot) + 128],
                    rhs=h3[:, bass.ds(hp_base + k, 1),
                           hp_col:hp_col + BL].rearrange("p a c -> p (a c)"),
                    start=False, stop=(k == 3), skip_group_check=True,
                )

        def rec_step(l, s, p2, q2, pa_emit):
            ps = state["ps"]
            hp_base = (q2 * 4) if s == 0 else (p2 * 4)
            hp_col = ((SC - 1) if s == 0 else (s - 1)) * BL
            hw = p2 * 4
            # PE order: k01 A, k23 A, k01 B, k23 B, then phase-A matmuls
            # (they fill the PE idle window while the chains run)
            for half in (0, 1):
                for pos in range(8):
                    rec_mm(l, ps, half, pos, (0, 1), hp_base, hp_col)
                for pos in range(8):
                    rec_mm(l, ps, half, pos, (2, 3), hp_base, hp_col)
            pa_ps = pa_emit(s) if pa_emit is not None else None
            # nonlinearity chains; half A first (its h gates the next step)
            gts, ths = [None, None], [None, None]
            for half in (0, 1):
                # psum half-bank layout: i|f|o|g x 64 cols
                gt = ew.tile([128, 256], F32, tag=f"gt{half}", name=f"gt{half}")
                gts[half] = gt
                nc.scalar.activation(gt[:, 0:128], ps[half][:, 0:128],
                                     AF.Sigmoid)                    # i, f
                nc.scalar.activation(gt[:, 192:256], ps[half][:, 192:256],
                                     AF.Tanh)                       # g
                # c update on DVE (c never leaves the vector engine)
                cs = c_t[:, half * 64:(half + 1) * 64]
                t1 = ew.tile([128, 64], F32, tag=f"t1{half}", name=f"t1_{half}")
                t2 = ew.tile([128, 64], F32, tag=f"t2{half}", name=f"t2_{half}")
                nc.vector.tensor_mul(t2[:], gt[:, 64:128], cs)      # f*c
                nc.vector.tensor_mul(t1[:], gt[:, 0:64],
                                     gt[:, 192:256])                # i*g
                nc.scalar.activation(gt[:, 128:192], ps[half][:, 128:192],
                                     AF.Sigmoid)                    # o (slack)
                nc.vector.tensor_add(cs, t1[:], t2[:])
                th = ew.tile([128, 64], F32, tag=f"th{half}", name=f"th{half}")
                ths[half] = th
                nc.scalar.activation(th[:], cs, AF.Tanh)
                nc.vector.tensor_mul(
                    h3[:, bass.ds(hw + 2 * half, 2), s * BL:(s + 1) * BL],
                    gt[:, 128:192].rearrange("p (a b) -> p a b", b=BL),
                    th[:].rearrange("p (a b) -> p a b", b=BL))
            # phase-A evac must be emitted BEFORE the chunk-boundary preload:
            # at s==15 the preload reads next-parity G slot 15, which this
            # step's evac writes (tile program order = emission order)
            if pa_ps is not None:
                pa_ps()
            # preload for next step (DVE, fills idle slots after the chains)
            if s == SC - 1:
                ps_next = alloc_preload(q2, 0)
            else:
                ps_next = alloc_preload(p2, s + 1)
            state["ps"] = ps_next

        for l in range(3):
            in_dram = xt.ap() if l == 0 else hdr[:]
            kc = KCS[l]

            # prologue: In chunks 0,1 -> bufs 0,1 ; G chunk 0 -> parity 0
            nc.sync.dma_start(i3[:, 0:kc, :], in_dram[:, :, 0:CW])
            nc.sync.dma_start(i3[:, kc:2 * kc, :], in_dram[:, :, CW:2 * CW])
            for slot in range(16):
                phase_a_slot(l, slot, 0, 0,
                             lambda idx: i3[:, bass.ds(idx, 1), :]
                             .rearrange("p a c -> p (a c)"))
            nc.vector.memset(c_t[:], 0.0)
            nc.vector.memset(h3[:, bass.ds(4, 4), (SC - 1) * BL: SC * BL], 0.0)
            state["ps"] = alloc_preload(0, 0)

            def chunk_body(iv, j, l=l, kc=kc, in_dram=in_dram):
                p2 = j & 1
                q2 = 1 - p2
                ld_buf = ((j + 2) & 3) * kc
                use_buf = ((j + 1) & 3) * kc
                nc.sync.dma_start(
                    i3[:, bass.ds(ld_buf, kc), :],
                    in_dram[:, :, bass.ds((iv + j + 2) * CW, CW)])

                def pa_emit(s, l=l, use_buf=use_buf, q2=q2):
                    slots = [s] if s < 14 else ([14, 15] if s == 14 else [])
                    pss = [(phase_a_mms(l, sl, use_buf,
                                        lambda idx: i3[:, bass.ds(idx, 1), :]
                                        .rearrange("p a c -> p (a c)")), sl)
                           for sl in slots]

                    def evac():
                        for ps, sl in pss:
                            phase_a_evac(ps, l, sl, q2 * 16)
                    return evac

                for s in range(SC):
                    rec_step(l, s, p2, q2, pa_emit)
                if l < 2:
                    nc.sync.dma_start(
                        hdr[:, :, bass.ds((iv + j) * CW, CW)],
                        h3[:, bass.ds(p2 * 4, 4), :])

            with tc.For_i(0, NCH, 4) as iv:
                for j in range(4):
                    chunk_body(iv, j)

        # final linear head: out.T [3, BL] = fcW @ h_last (+ fcB)
        hb = ((NCH - 1) & 1) * 4
        ps = ps_fc.tile([3, BL], F32, tag="fc")
        for k in range(4):
            nc.tensor.matmul(
                ps[:], lhsT=fcw_sb[:, k * 3:(k + 1) * 3],
                rhs=h3[:, bass.ds(hb + k, 1), (SC - 1) * BL: SC * BL]
                .rearrange("p a c -> p (a c)"),
                start=(k == 0), stop=(k == 3))
        ob = ew.tile([3, BL], F32, tag="out")
        nc.scalar.activation(ob[:], ps[:], AF.Identity, bias=fcb_sb[:])
        nc.sync.dma_start(out_d.ap(), ob[:])

    nc.compile()
    return nc


def _prep(inputs):
    """Host-side layout prep. Returns per-core in_maps."""
    bf = ml_dtypes.bfloat16
    x = np.asarray(inputs["x"], np.float32)
    wihs = [np.asarray(inputs[f"Wih{l}"], np.float32) for l in range(3)]
    whhs = [np.asarray(inputs[f"Whh{l}"], np.float32) for l in range(3)]

    def wt_pack(w, kcs):  # [2048, K] -> [128, kcs*2048]
        return np.ascontiguousarray(
            w.T.reshape(kcs, 128, 2048).transpose(1, 0, 2)
            .reshape(128, kcs * 2048)).astype(bf)

    shared = {}
    for l in range(3):
        shared[f"wih{l}"] = wt_pack(wihs[l], KCS[l])
        shared[f"whh{l}"] = wt_pack(whhs[l], 4)
    bias = np.zeros((128, 48), np.float32)
    for l in range(3):
        bl_ = (np.asarray(inputs[f"bih{l}"], np.float32)
               + np.asarray(inputs[f"bhh{l}"], np.float32))
        for slot in range(16):
            r = _slot_row(slot)
            bias[:, l * 16 + slot] = bl_[r:r + 128]
    shared["bias"] = bias
    shared["fcw"] = np.ascontiguousarray(
        np.asarray(inputs["fcW"], np.float32).T.reshape(4, 128, 3)
        .transpose(1, 0, 2).reshape(128, 12)).astype(bf)
    shared["fcb"] = np.asarray(inputs["fcB"], np.float32).reshape(3, 1)

    in_maps = []
    for c in range(NCORES):
        xc = x[c * BL:(c + 1) * BL]                       # [32, 512, 256]
        xp = xc.transpose(2, 1, 0).reshape(2, 128, TOT)   # [2,128,16384]
        xp = np.ascontiguousarray(xp.transpose(1, 0, 2))  # [128,2,16384]
        xp = np.concatenate(
            [xp, np.zeros((128, 2, SLACK), np.float32)], axis=2).astype(bf)
        in_maps.append({"x_t": xp, **shared})
    return in_maps


_NC_CACHE = None


def kernel(**inputs):
    global _NC_CACHE
    if _NC_CACHE is None:
        _NC_CACHE = _build()
    nc = _NC_CACHE
    in_maps = _prep(inputs)
    res = run_bass_kernel_spmd(nc, in_maps, core_ids=list(range(NCORES)))
    out = np.empty((B, O), np.float32)
    for c in range(NCORES):
        out[c * BL:(c + 1) * BL] = res.results[c]["out"].T
    return out
